# revision 15
# baseline (speedup 1.0000x reference)
"""Trainium2 Bass kernel for dual-attention (DisKT-style) nn module.

Math per (batch, head) with S=1024, dk=64, on-chip in [k, q] layout:
    sT       = (k_h @ q_h^T)          (+ -1e30 on causal-dead diag block)
    E1T      = exp(sT / 8)            (bf16; causally-dead region = 0)
    r1[q]    = sum_k E1T[k, q]        (ones^T @ E1T, PSUM broadcast rows)
    p1       = E1T * rec1[q]          (bf16, DVE 2x mode)
    E2''     = exp(p1) - 1            (bf16; "+1" of every key becomes a
                                       rank-1 vtot fixup applied on host)
    ot1      = [cm*v1 | cm]^T @ E2''  (M=65: row 64 accumulates r2 for free)
    ot2      = [cm*v2]^T   @ E2''
Host: out = (ot[0:64] + vtot) / (1024 + r2);  out[q=0] = 0; transpose.

The counter-mask is folded into the PV weights host-side; the causal-dead
packed layout keeps exp/mul/sub as few big instructions.  Emission is a
2-block-skew software pipeline interleaved at QK-group granularity:
    QK(n,Gi) | PV(n-2, chunk pair i) | exp2/sub(n-1 slotted between groups)
so the in-order PE stream stays continuously fed (p-state stays high) while
ACT (exp1+exp2) runs ~1 group ahead of the PE consumers.

Sharding: data-parallel over batch, B=16 -> 2 per core on 8 cores.
"""

import numpy as np
import ml_dtypes

import concourse.bass as bass
import concourse.mybir as mybir
import concourse.tile as tile
from concourse import bacc
from concourse.bass_utils import run_bass_kernel_spmd

B, S, D, H = 16, 1024, 512, 8
DK = D // H           # 64
NCORES = 8
BLOC = B // NCORES    # 2 batches per core
NCH = S // 128        # 8 k-chunks of 128
F32 = mybir.dt.float32
BF16 = mybir.dt.bfloat16
NPBF16 = ml_dtypes.bfloat16

LIVE = [S - 128 * c for c in range(NCH)]          # live width per chunk
OFF = [sum(LIVE[:c]) for c in range(NCH)]         # packed offset per chunk
PACK = OFF[-1] + LIVE[-1]                         # 4608
# chunk groups sharing one scores-psum tile + one exp1 instruction
GROUPS = [[0], [1], [2], [3], [4, 5], [6, 7]]
NG = len(GROUPS)
# split of the packed exp2/sub into two instructions (chunks 0-1 | 2-7)
CSPLIT = OFF[2]

# knobs that test.py can flip
TRACE = False
LAST_RESULTS = None


def bank_pieces(p0, p1):
    """split [p0, p1) at 512-aligned psum bank boundaries"""
    out = []
    p = p0
    while p < p1:
        end = min(p1, (p // 512 + 1) * 512)
        out.append((p, end))
        p = end
    return out


def build_nc(debug=False):
    nc = bacc.Bacc("TRN2", target_bir_lowering=False, debug=debug)
    AF = mybir.ActivationFunctionType

    qt_d = nc.dram_tensor("qt", [BLOC, H, DK, S], BF16, kind="ExternalInput")
    kt_d = nc.dram_tensor("kt", [BLOC, H, DK, S], BF16, kind="ExternalInput")
    # PV weights, host-transposed to [keys, chunk, dims] per (b, h):
    # w1 dims 0-63 = cm*v1, dim 64 = cm (accumulates r2 in psum row 64)
    w1_d = nc.dram_tensor("w1", [BLOC, H, 128, NCH, 65], BF16, kind="ExternalInput")
    w2_d = nc.dram_tensor("w2", [BLOC, H, 128, NCH, 64], BF16, kind="ExternalInput")
    dmask_d = nc.dram_tensor("dmask", [128, 128], BF16, kind="ExternalInput")
    ident_d = nc.dram_tensor("ident", [128, 128], BF16, kind="ExternalInput")
    ones_d = nc.dram_tensor("onesd", [128, 128], BF16, kind="ExternalInput")
    # raw outputs: [65|64, q] bf16 per (b, h); host normalizes + transposes
    o1_d = nc.dram_tensor("o1t", [BLOC, H, 65, S], BF16, kind="ExternalOutput")
    o2_d = nc.dram_tensor("o2t", [BLOC, H, 64, S], BF16, kind="ExternalOutput")

    with tile.TileContext(nc) as tc:
        with (
            tc.tile_pool(name="consts", bufs=1) as consts,
            tc.tile_pool(name="qk", bufs=4) as qkp,
            tc.tile_pool(name="w", bufs=4) as wp,
            tc.tile_pool(name="e1", bufs=3) as e1p,
            tc.tile_pool(name="tmp", bufs=2) as tmpp,
            tc.tile_pool(name="e2", bufs=4) as e2p,
            tc.tile_pool(name="rc", bufs=2) as rcp,
            tc.tile_pool(name="outs", bufs=2) as outp,
            tc.tile_pool(name="sc_ps", bufs=1, space="PSUM") as sc_psp,
            tc.tile_pool(name="r_ps", bufs=1, space="PSUM") as r_psp,
            tc.tile_pool(name="o1_ps", bufs=1, space="PSUM") as o1_psp,
            tc.tile_pool(name="o2_ps", bufs=1, space="PSUM") as o2_psp,
        ):
            dm_sb = consts.tile([128, 128], BF16)
            nc.sync.dma_start(out=dm_sb, in_=dmask_d[:, :])
            id_sb = consts.tile([128, 128], BF16)
            nc.sync.dma_start(out=id_sb, in_=ident_d[:, :])
            ones_sb = consts.tile([128, 128], BF16)
            nc.sync.dma_start(out=ones_sb, in_=ones_d[:, :])

            NB = BLOC * H
            st = [dict() for _ in range(NB)]

            def dma_in(blk):
                bi, h = divmod(blk, H)
                s = st[blk]
                s["qt"] = qkp.tile([DK, S], BF16, tag="qt", name="qt_sb")
                s["kt"] = qkp.tile([DK, S], BF16, tag="kt", name="kt_sb")
                nc.sync.dma_start(out=s["qt"], in_=qt_d[bi, h])
                nc.sync.dma_start(out=s["kt"], in_=kt_d[bi, h])
                s["w1"] = wp.tile([128, NCH, 65], BF16, tag="w1", name="w1_sb")
                s["w2"] = wp.tile([128, NCH, 64], BF16, tag="w2", name="w2_sb")
                nc.sync.dma_start(out=s["w1"], in_=w1_d[bi, h])
                nc.sync.dma_start(out=s["w2"], in_=w2_d[bi, h])

            def qk_group(blk, gi):
                """scores + causal mask -> exp1 (packed e1) for one group"""
                s = st[blk]
                grp = GROUPS[gi]
                gw = sum(LIVE[c] for c in grp)
                if gi == 0:
                    s["e1"] = e1p.tile([128, PACK], BF16, tag="e1", name="e1_sb")
                    s["r1ps"] = r_psp.tile([128, S], F32, tag="r1", name="r1ps")
                sps = sc_psp.tile([128, gw], F32, tag="sc", name="sps")
                s[f"sc{gi}"] = sps
                loc = 0
                for c in grp:
                    q0 = 128 * c
                    for n0 in range(0, LIVE[c], 512):
                        w = min(512, LIVE[c] - n0)
                        nc.tensor.matmul(
                            sps[:, loc + n0 : loc + n0 + w],
                            lhsT=s["kt"][:, q0 : q0 + 128],
                            rhs=s["qt"][:, q0 + n0 : q0 + n0 + w],
                            start=True,
                            stop=False,
                            skip_group_check=True,
                        )
                    # causal: += I^T @ dmask adds -1e30 on/above diag
                    nc.tensor.matmul(
                        sps[:, loc : loc + 128],
                        lhsT=id_sb,
                        rhs=dm_sb,
                        start=False,
                        stop=True,
                        skip_group_check=True,
                    )
                    loc += LIVE[c]
                o0 = OFF[grp[0]]
                nc.scalar.activation(
                    s["e1"][:, o0 : o0 + gw], sps[:, 0:gw], AF.Exp, scale=0.125
                )

            def r1_group(blk, gi):
                """r1 accumulation for one group's chunks"""
                s = st[blk]
                for c in GROUPS[gi]:
                    q0 = 128 * c
                    for p0, p1 in bank_pieces(q0, S):
                        nc.tensor.matmul(
                            s["r1ps"][:, p0:p1],
                            lhsT=ones_sb,
                            rhs=s["e1"][:, OFF[c] + p0 - q0 : OFF[c] + p1 - q0],
                            start=(c == 0),
                            stop=(c == NCH - 1),
                            skip_group_check=True,
                        )

            def rec1(blk):
                s = st[blk]
                rec1f = rcp.tile([128, S], F32, tag="rec1f")
                nc.vector.reciprocal_approx_fast(out=rec1f, in_=s["r1ps"][:, 0:S])
                rec1b = rcp.tile([128, S], BF16, tag="rec1b")
                nc.vector.tensor_copy(rec1b, rec1f)
                nc.vector.memset(rec1b[:, 0:1], 0.0)
                s["rec1"] = rec1b

            def muls(blk):
                """p1 = e1 * rec1 (bf16, DVE 2x) into tmp"""
                s = st[blk]
                s["tmp"] = tmpp.tile([128, PACK], BF16, tag="tmp", name="tmp_sb")
                for c in range(NCH):
                    q0 = 128 * c
                    nc.vector.tensor_mul(
                        s["tmp"][:, OFF[c] : OFF[c] + LIVE[c]],
                        s["e1"][:, OFF[c] : OFF[c] + LIVE[c]],
                        s["rec1"][:, q0:S],
                    )

            def exp2_part(blk, half):
                s = st[blk]
                x0, x1 = (0, CSPLIT) if half == 0 else (CSPLIT, PACK)
                nc.scalar.activation(s["tmp"][:, x0:x1], s["tmp"][:, x0:x1], AF.Exp)

            def sub_part(blk, half):
                # on gpsimd: idle engine, and the 3-block skew gives the
                # e2 chain ~2 periods of slack before PV consumes it
                s = st[blk]
                if half == 0:
                    s["e2"] = e2p.tile([128, PACK], BF16, tag="e2", name="e2_sb")
                x0, x1 = (0, CSPLIT) if half == 0 else (CSPLIT, PACK)
                nc.gpsimd.tensor_scalar_add(
                    s["e2"][:, x0:x1], s["tmp"][:, x0:x1], -1.0
                )

            def pv_pair(blk, gi):
                """dense PV sweep for this group's chunks (both outputs)"""
                s = st[blk]
                if gi == 0:
                    s["ot1"] = o1_psp.tile([65, S], F32, tag="ot1", name="ot1ps")
                    s["ot2"] = o2_psp.tile([64, S], F32, tag="ot2", name="ot2ps")
                for c in GROUPS[gi]:
                    q0 = 128 * c
                    for p0, p1 in bank_pieces(q0, S):
                        ee = s["e2"][:, OFF[c] + p0 - q0 : OFF[c] + p1 - q0]
                        nc.tensor.matmul(
                            s["ot1"][:, p0:p1],
                            lhsT=s["w1"][:, c, :],
                            rhs=ee,
                            start=(c == 0),
                            stop=(c == NCH - 1),
                            skip_group_check=True,
                        )
                        nc.tensor.matmul(
                            s["ot2"][:, p0:p1],
                            lhsT=s["w2"][:, c, :],
                            rhs=ee,
                            start=(c == 0),
                            stop=(c == NCH - 1),
                            skip_group_check=True,
                        )

            def dma_out(blk):
                """psum -> sbuf bf16 on DVE (DMA can't read PSUM), then out"""
                bi, h = divmod(blk, H)
                s = st[blk]
                o1_sb = outp.tile([65, S], BF16, tag="o1", name="o1_sb")
                o2_sb = outp.tile([64, S], BF16, tag="o2", name="o2_sb")
                nc.vector.tensor_copy(o1_sb, s["ot1"][:, :])
                nc.vector.tensor_copy(o2_sb, s["ot2"][:, :])
                nc.sync.dma_start(out=o1_d[bi, h], in_=o1_sb)
                nc.sync.dma_start(out=o2_d[bi, h], in_=o2_sb)

            # 3-block-skew pipeline, interleaved at group granularity:
            #   A(n) QK/exp1/r1 | C(n-1) mul/exp2/sub | D(n-3) PV
            # exp1s always lead the ACT queue; exp2 parts slot in behind
            # the exp1 their consumers don't need yet.  PV reads e2 two
            # full periods after the subs, so the elementwise chain is
            # never on the PE critical path.
            dma_in(0)
            for n in range(NB + 3):
                vA = n < NB
                vC = 1 <= n <= NB
                vD = n >= 3
                if n + 1 < NB:
                    dma_in(n + 1)
                if vC:
                    muls(n - 1)
                for gi in range(NG):
                    if vA:
                        qk_group(n, gi)
                        if gi >= 2:
                            r1_group(n, gi - 2)
                    if vD:
                        pv_pair(n - 3, gi)
                    if vC and gi == 2:
                        exp2_part(n - 1, 0)
                    if vC and gi == 3:
                        sub_part(n - 1, 0)
                    if vC and gi == 4:
                        exp2_part(n - 1, 1)
                    if vC and gi == 5:
                        sub_part(n - 1, 1)
                if vA:
                    r1_group(n, NG - 2)
                    r1_group(n, NG - 1)
                    rec1(n)
                if vD:
                    dma_out(n - 3)

    nc.compile()
    return nc


_NC_CACHE = None


def _get_nc():
    global _NC_CACHE
    if _NC_CACHE is None:
        _NC_CACHE = build_nc()
    return _NC_CACHE


def make_in_maps(q, k, v1, v2, cm):
    """Full inputs -> per-core input maps (host-side sharding + layout)."""
    q = np.asarray(q, dtype=np.float32).astype(NPBF16)
    k = np.asarray(k, dtype=np.float32).astype(NPBF16)
    v1 = np.asarray(v1, dtype=np.float32)
    v2 = np.asarray(v2, dtype=np.float32)
    cm = np.asarray(cm)

    # additive causal mask for the diagonal block: 0 where k < q else -1e30
    dmask = np.where(
        np.arange(128)[:, None] < np.arange(128)[None, :], 0.0, -1e30
    ).astype(NPBF16)
    ident = np.eye(128, dtype=NPBF16)
    onesd = np.ones((128, 128), NPBF16)

    cml = 1.0 - cm.astype(np.float32)  # [B, S]
    vcm1 = v1 * cml[:, :, None]
    vcm2 = v2 * cml[:, :, None]

    in_maps = []
    for core in range(NCORES):
        b0 = core * BLOC
        qt = np.ascontiguousarray(
            q[b0 : b0 + BLOC].reshape(BLOC, S, H, DK).transpose(0, 2, 3, 1)
        )  # [b, h, dk, s]
        kt = np.ascontiguousarray(
            k[b0 : b0 + BLOC].reshape(BLOC, S, H, DK).transpose(0, 2, 3, 1)
        )
        # [b, h, key-in-chunk, chunk, dim] weight layout (contiguous per key)
        v1s = (
            vcm1[b0 : b0 + BLOC]
            .reshape(BLOC, NCH, 128, H, DK)
            .transpose(0, 3, 2, 1, 4)
        )  # [b, h, 128, c, dk]
        v2s = (
            vcm2[b0 : b0 + BLOC]
            .reshape(BLOC, NCH, 128, H, DK)
            .transpose(0, 3, 2, 1, 4)
        )
        cmls = np.broadcast_to(
            cml[b0 : b0 + BLOC].reshape(BLOC, 1, NCH, 128).transpose(0, 1, 3, 2)[
                :, :, :, :, None
            ],
            (BLOC, H, 128, NCH, 1),
        )
        w1 = np.ascontiguousarray(
            np.concatenate([v1s, cmls], axis=-1).astype(NPBF16)
        )  # [b, h, 128, c, 65]
        w2 = np.ascontiguousarray(v2s.astype(NPBF16))
        in_maps.append(
            dict(qt=qt, kt=kt, w1=w1, w2=w2, dmask=dmask, ident=ident, onesd=onesd)
        )
    return in_maps


def _finish(res, v1, v2):
    o1 = np.concatenate(
        [r["o1t"].astype(np.float32) for r in res.results], axis=0
    )  # [B, H, 65, S]
    o2 = np.concatenate([r["o2t"].astype(np.float32) for r in res.results], axis=0)
    vtot1 = (
        v1.astype(np.float64).reshape(B, S, H, DK).sum(axis=1).astype(np.float32)
    )  # [B, H, dk]
    vtot2 = v2.astype(np.float64).reshape(B, S, H, DK).sum(axis=1).astype(np.float32)
    r2 = 1024.0 + o1[:, :, 64, :]  # [B, H, S]
    out1 = (o1[:, :, 0:64, :] + vtot1[:, :, :, None]) / r2[:, :, None, :]
    out2 = (o2 + vtot2[:, :, :, None]) / r2[:, :, None, :]
    out1 = out1.transpose(0, 3, 1, 2).reshape(B, S, D)
    out2 = out2.transpose(0, 3, 1, 2).reshape(B, S, D)
    out1[:, 0, :] = 0.0
    out2[:, 0, :] = 0.0
    return np.ascontiguousarray(out1), np.ascontiguousarray(out2)


def kernel(q, k, v1, v2, counter_attention_mask):
    global LAST_RESULTS
    v1 = np.asarray(v1, dtype=np.float32)
    v2 = np.asarray(v2, dtype=np.float32)
    in_maps = make_in_maps(q, k, v1, v2, counter_attention_mask)
    nc = _get_nc()
    res = run_bass_kernel_spmd(
        nc, in_maps, core_ids=list(range(NCORES)), trace=TRACE
    )
    LAST_RESULTS = res
    return _finish(res, v1, v2)


# revision 16
# speedup vs baseline: 4.1304x; 4.1304x over previous
"""Trainium2 Bass kernel for dual-attention (DisKT-style) nn module.

Math per (batch, head) with S=1024, dk=64, on-chip in [k, q] layout:
    sT       = (k_h @ q_h^T)          (+ -1e30 on causal-dead diag block)
    E1T      = exp(sT / 8)            (bf16; causally-dead region = 0)
    r1[q]    = sum_k E1T[k, q]        (ones^T @ E1T, PSUM broadcast rows)
    p1       = E1T * rec1[q]          (bf16, DVE 2x mode)
    E2''     = exp(p1) - 1            (bf16; "+1" of every key becomes a
                                       rank-1 vtot fixup applied on host)
    ot1      = [cm*v1 | cm]^T @ E2''  (M=65: row 64 accumulates r2 for free)
    ot2      = [cm*v2]^T   @ E2''
Host: out = (ot[0:64] + vtot) / (1024 + r2);  out[q=0] = 0; transpose.

The counter-mask is folded into the PV weights host-side; the causal-dead
packed layout keeps exp/mul/sub as few big instructions.  Emission is a
2-block-skew software pipeline interleaved at QK-group granularity:
    QK(n,Gi) | PV(n-2, chunk pair i) | exp2/sub(n-1 slotted between groups)
so the in-order PE stream stays continuously fed (p-state stays high) while
ACT (exp1+exp2) runs ~1 group ahead of the PE consumers.

Sharding: data-parallel over batch, B=16 -> 2 per core on 8 cores.
"""

import numpy as np
import ml_dtypes

import concourse.bass as bass
import concourse.mybir as mybir
import concourse.tile as tile
from concourse import bacc
from concourse.bass_utils import run_bass_kernel_spmd

B, S, D, H = 16, 1024, 512, 8
DK = D // H           # 64
NCORES = 8
BLOC = B // NCORES    # 2 batches per core
NCH = S // 128        # 8 k-chunks of 128
F32 = mybir.dt.float32
BF16 = mybir.dt.bfloat16
NPBF16 = ml_dtypes.bfloat16

LIVE = [S - 128 * c for c in range(NCH)]          # live width per chunk
OFF = [sum(LIVE[:c]) for c in range(NCH)]         # packed offset per chunk
PACK = OFF[-1] + LIVE[-1]                         # 4608
# chunk groups sharing one scores-psum tile + one exp1 instruction
GROUPS = [[0], [1], [2], [3], [4, 5], [6, 7]]
NG = len(GROUPS)
# split of the packed exp2/sub into two instructions (chunks 0-1 | 2-7)
CSPLIT = OFF[2]

# knobs that test.py can flip
TRACE = False
LAST_RESULTS = None


def bank_pieces(p0, p1):
    """split [p0, p1) at 512-aligned psum bank boundaries"""
    out = []
    p = p0
    while p < p1:
        end = min(p1, (p // 512 + 1) * 512)
        out.append((p, end))
        p = end
    return out


def build_nc(debug=False):
    nc = bacc.Bacc("TRN2", target_bir_lowering=False, debug=debug)
    AF = mybir.ActivationFunctionType

    qt_d = nc.dram_tensor("qt", [BLOC, H, DK, S], BF16, kind="ExternalInput")
    kt_d = nc.dram_tensor("kt", [BLOC, H, DK, S], BF16, kind="ExternalInput")
    # PV weights, host-transposed to [keys, chunk, dims] per (b, h):
    # w1 dims 0-63 = cm*v1, dim 64 = cm (accumulates r2 in psum row 64)
    w1_d = nc.dram_tensor("w1", [BLOC, H, 128, NCH, 65], BF16, kind="ExternalInput")
    w2_d = nc.dram_tensor("w2", [BLOC, H, 128, NCH, 64], BF16, kind="ExternalInput")
    dmask_d = nc.dram_tensor("dmask", [128, 128], BF16, kind="ExternalInput")
    ident_d = nc.dram_tensor("ident", [128, 128], BF16, kind="ExternalInput")
    ones_d = nc.dram_tensor("onesd", [128, 128], BF16, kind="ExternalInput")
    # raw outputs: [65|64, q] bf16 per (b, h); host normalizes + transposes
    o1_d = nc.dram_tensor("o1t", [BLOC, H, 65, S], BF16, kind="ExternalOutput")
    o2_d = nc.dram_tensor("o2t", [BLOC, H, 64, S], BF16, kind="ExternalOutput")

    with tile.TileContext(nc) as tc:
        with (
            tc.tile_pool(name="consts", bufs=1) as consts,
            tc.tile_pool(name="qk", bufs=4) as qkp,
            tc.tile_pool(name="w", bufs=4) as wp,
            tc.tile_pool(name="e1", bufs=3) as e1p,
            tc.tile_pool(name="tmp", bufs=2) as tmpp,
            tc.tile_pool(name="e2", bufs=4) as e2p,
            tc.tile_pool(name="rc", bufs=2) as rcp,
            tc.tile_pool(name="outs", bufs=2) as outp,
            tc.tile_pool(name="sc_ps", bufs=1, space="PSUM") as sc_psp,
            tc.tile_pool(name="r_ps", bufs=1, space="PSUM") as r_psp,
            tc.tile_pool(name="o1_ps", bufs=1, space="PSUM") as o1_psp,
            tc.tile_pool(name="o2_ps", bufs=1, space="PSUM") as o2_psp,
        ):
            dm_sb = consts.tile([128, 128], BF16)
            nc.sync.dma_start(out=dm_sb, in_=dmask_d[:, :])
            id_sb = consts.tile([128, 128], BF16)
            nc.sync.dma_start(out=id_sb, in_=ident_d[:, :])
            ones_sb = consts.tile([128, 128], BF16)
            nc.sync.dma_start(out=ones_sb, in_=ones_d[:, :])

            NB = BLOC * H
            st = [dict() for _ in range(NB)]

            def dma_in(blk):
                bi, h = divmod(blk, H)
                s = st[blk]
                s["qt"] = qkp.tile([DK, S], BF16, tag="qt", name="qt_sb")
                s["kt"] = qkp.tile([DK, S], BF16, tag="kt", name="kt_sb")
                nc.sync.dma_start(out=s["qt"], in_=qt_d[bi, h])
                nc.sync.dma_start(out=s["kt"], in_=kt_d[bi, h])
                s["w1"] = wp.tile([128, NCH, 65], BF16, tag="w1", name="w1_sb")
                s["w2"] = wp.tile([128, NCH, 64], BF16, tag="w2", name="w2_sb")
                nc.sync.dma_start(out=s["w1"], in_=w1_d[bi, h])
                nc.sync.dma_start(out=s["w2"], in_=w2_d[bi, h])

            def qk_group(blk, gi):
                """scores + causal mask -> exp1 (packed e1) for one group"""
                s = st[blk]
                grp = GROUPS[gi]
                gw = sum(LIVE[c] for c in grp)
                if gi == 0:
                    s["e1"] = e1p.tile([128, PACK], BF16, tag="e1", name="e1_sb")
                    s["r1ps"] = r_psp.tile([128, S], F32, tag="r1", name="r1ps")
                sps = sc_psp.tile([128, gw], F32, tag="sc", name="sps")
                s[f"sc{gi}"] = sps
                loc = 0
                for c in grp:
                    q0 = 128 * c
                    for n0 in range(0, LIVE[c], 512):
                        w = min(512, LIVE[c] - n0)
                        nc.tensor.matmul(
                            sps[:, loc + n0 : loc + n0 + w],
                            lhsT=s["kt"][:, q0 : q0 + 128],
                            rhs=s["qt"][:, q0 + n0 : q0 + n0 + w],
                            start=True,
                            stop=False,
                            skip_group_check=True,
                        )
                    # causal: += I^T @ dmask adds -1e30 on/above diag
                    nc.tensor.matmul(
                        sps[:, loc : loc + 128],
                        lhsT=id_sb,
                        rhs=dm_sb,
                        start=False,
                        stop=True,
                        skip_group_check=True,
                    )
                    loc += LIVE[c]
                o0 = OFF[grp[0]]
                nc.scalar.activation(
                    s["e1"][:, o0 : o0 + gw], sps[:, 0:gw], AF.Exp, scale=0.125
                )

            def r1_group(blk, gi):
                """r1 accumulation for one group's chunks"""
                s = st[blk]
                for c in GROUPS[gi]:
                    q0 = 128 * c
                    for p0, p1 in bank_pieces(q0, S):
                        nc.tensor.matmul(
                            s["r1ps"][:, p0:p1],
                            lhsT=ones_sb,
                            rhs=s["e1"][:, OFF[c] + p0 - q0 : OFF[c] + p1 - q0],
                            start=(c == 0),
                            stop=(c == NCH - 1),
                            skip_group_check=True,
                        )

            def rec1(blk):
                s = st[blk]
                rec1f = rcp.tile([128, S], F32, tag="rec1f")
                nc.vector.reciprocal_approx_fast(out=rec1f, in_=s["r1ps"][:, 0:S])
                rec1b = rcp.tile([128, S], BF16, tag="rec1b")
                nc.vector.tensor_copy(rec1b, rec1f)
                nc.vector.memset(rec1b[:, 0:1], 0.0)
                s["rec1"] = rec1b

            def muls(blk):
                """p1 = e1 * rec1 (bf16, DVE 2x) into tmp"""
                s = st[blk]
                s["tmp"] = tmpp.tile([128, PACK], BF16, tag="tmp", name="tmp_sb")
                for c in range(NCH):
                    q0 = 128 * c
                    nc.vector.tensor_mul(
                        s["tmp"][:, OFF[c] : OFF[c] + LIVE[c]],
                        s["e1"][:, OFF[c] : OFF[c] + LIVE[c]],
                        s["rec1"][:, q0:S],
                    )

            def exp2_part(blk, half):
                s = st[blk]
                x0, x1 = (0, CSPLIT) if half == 0 else (CSPLIT, PACK)
                nc.scalar.activation(s["tmp"][:, x0:x1], s["tmp"][:, x0:x1], AF.Exp)

            def sub_part(blk, half):
                s = st[blk]
                if half == 0:
                    s["e2"] = e2p.tile([128, PACK], BF16, tag="e2", name="e2_sb")
                x0, x1 = (0, CSPLIT) if half == 0 else (CSPLIT, PACK)
                nc.vector.tensor_scalar_add(
                    s["e2"][:, x0:x1], s["tmp"][:, x0:x1], -1.0
                )

            def pv_pair(blk, gi):
                """dense PV sweep for this group's chunks (both outputs)"""
                s = st[blk]
                if gi == 0:
                    s["ot1"] = o1_psp.tile([65, S], F32, tag="ot1", name="ot1ps")
                    s["ot2"] = o2_psp.tile([64, S], F32, tag="ot2", name="ot2ps")
                for c in GROUPS[gi]:
                    q0 = 128 * c
                    for p0, p1 in bank_pieces(q0, S):
                        ee = s["e2"][:, OFF[c] + p0 - q0 : OFF[c] + p1 - q0]
                        nc.tensor.matmul(
                            s["ot1"][:, p0:p1],
                            lhsT=s["w1"][:, c, :],
                            rhs=ee,
                            start=(c == 0),
                            stop=(c == NCH - 1),
                            skip_group_check=True,
                        )
                        nc.tensor.matmul(
                            s["ot2"][:, p0:p1],
                            lhsT=s["w2"][:, c, :],
                            rhs=ee,
                            start=(c == 0),
                            stop=(c == NCH - 1),
                            skip_group_check=True,
                        )

            def dma_out(blk):
                """psum -> sbuf bf16 on DVE (DMA can't read PSUM), then out"""
                bi, h = divmod(blk, H)
                s = st[blk]
                o1_sb = outp.tile([65, S], BF16, tag="o1", name="o1_sb")
                o2_sb = outp.tile([64, S], BF16, tag="o2", name="o2_sb")
                nc.vector.tensor_copy(o1_sb, s["ot1"][:, :])
                nc.vector.tensor_copy(o2_sb, s["ot2"][:, :])
                nc.sync.dma_start(out=o1_d[bi, h], in_=o1_sb)
                nc.sync.dma_start(out=o2_d[bi, h], in_=o2_sb)

            # 3-block-skew pipeline, interleaved at group granularity:
            #   A(n) QK/exp1/r1 | C(n-1) mul/exp2/sub | D(n-3) PV
            # exp1s always lead the ACT queue; exp2 parts slot in behind
            # the exp1 their consumers don't need yet.  PV reads e2 two
            # full periods after the subs, so the elementwise chain is
            # never on the PE critical path.
            dma_in(0)
            for n in range(NB + 3):
                vA = n < NB
                vC = 1 <= n <= NB
                vD = n >= 3
                if n + 1 < NB:
                    dma_in(n + 1)
                if vC:
                    muls(n - 1)
                for gi in range(NG):
                    if vA:
                        qk_group(n, gi)
                        if gi >= 2:
                            r1_group(n, gi - 2)
                    if vD:
                        pv_pair(n - 3, gi)
                    if vC and gi == 2:
                        exp2_part(n - 1, 0)
                    if vC and gi == 3:
                        sub_part(n - 1, 0)
                    if vC and gi == 4:
                        exp2_part(n - 1, 1)
                    if vC and gi == 5:
                        sub_part(n - 1, 1)
                if vA:
                    r1_group(n, NG - 2)
                    r1_group(n, NG - 1)
                    rec1(n)
                if vD:
                    dma_out(n - 3)

    nc.compile()
    return nc


_NC_CACHE = None


def _get_nc():
    global _NC_CACHE
    if _NC_CACHE is None:
        _NC_CACHE = build_nc()
    return _NC_CACHE


def make_in_maps(q, k, v1, v2, cm):
    """Full inputs -> per-core input maps (host-side sharding + layout)."""
    q = np.asarray(q, dtype=np.float32).astype(NPBF16)
    k = np.asarray(k, dtype=np.float32).astype(NPBF16)
    v1 = np.asarray(v1, dtype=np.float32)
    v2 = np.asarray(v2, dtype=np.float32)
    cm = np.asarray(cm)

    # additive causal mask for the diagonal block: 0 where k < q else -1e30
    dmask = np.where(
        np.arange(128)[:, None] < np.arange(128)[None, :], 0.0, -1e30
    ).astype(NPBF16)
    ident = np.eye(128, dtype=NPBF16)
    onesd = np.ones((128, 128), NPBF16)

    cml = 1.0 - cm.astype(np.float32)  # [B, S]
    vcm1 = v1 * cml[:, :, None]
    vcm2 = v2 * cml[:, :, None]

    in_maps = []
    for core in range(NCORES):
        b0 = core * BLOC
        qt = np.ascontiguousarray(
            q[b0 : b0 + BLOC].reshape(BLOC, S, H, DK).transpose(0, 2, 3, 1)
        )  # [b, h, dk, s]
        kt = np.ascontiguousarray(
            k[b0 : b0 + BLOC].reshape(BLOC, S, H, DK).transpose(0, 2, 3, 1)
        )
        # [b, h, key-in-chunk, chunk, dim] weight layout (contiguous per key)
        v1s = (
            vcm1[b0 : b0 + BLOC]
            .reshape(BLOC, NCH, 128, H, DK)
            .transpose(0, 3, 2, 1, 4)
        )  # [b, h, 128, c, dk]
        v2s = (
            vcm2[b0 : b0 + BLOC]
            .reshape(BLOC, NCH, 128, H, DK)
            .transpose(0, 3, 2, 1, 4)
        )
        cmls = np.broadcast_to(
            cml[b0 : b0 + BLOC].reshape(BLOC, 1, NCH, 128).transpose(0, 1, 3, 2)[
                :, :, :, :, None
            ],
            (BLOC, H, 128, NCH, 1),
        )
        w1 = np.ascontiguousarray(
            np.concatenate([v1s, cmls], axis=-1).astype(NPBF16)
        )  # [b, h, 128, c, 65]
        w2 = np.ascontiguousarray(v2s.astype(NPBF16))
        in_maps.append(
            dict(qt=qt, kt=kt, w1=w1, w2=w2, dmask=dmask, ident=ident, onesd=onesd)
        )
    return in_maps


def _finish(res, v1, v2):
    o1 = np.concatenate(
        [r["o1t"].astype(np.float32) for r in res.results], axis=0
    )  # [B, H, 65, S]
    o2 = np.concatenate([r["o2t"].astype(np.float32) for r in res.results], axis=0)
    vtot1 = (
        v1.astype(np.float64).reshape(B, S, H, DK).sum(axis=1).astype(np.float32)
    )  # [B, H, dk]
    vtot2 = v2.astype(np.float64).reshape(B, S, H, DK).sum(axis=1).astype(np.float32)
    r2 = 1024.0 + o1[:, :, 64, :]  # [B, H, S]
    out1 = (o1[:, :, 0:64, :] + vtot1[:, :, :, None]) / r2[:, :, None, :]
    out2 = (o2 + vtot2[:, :, :, None]) / r2[:, :, None, :]
    out1 = out1.transpose(0, 3, 1, 2).reshape(B, S, D)
    out2 = out2.transpose(0, 3, 1, 2).reshape(B, S, D)
    out1[:, 0, :] = 0.0
    out2[:, 0, :] = 0.0
    return np.ascontiguousarray(out1), np.ascontiguousarray(out2)


def kernel(q, k, v1, v2, counter_attention_mask):
    global LAST_RESULTS
    v1 = np.asarray(v1, dtype=np.float32)
    v2 = np.asarray(v2, dtype=np.float32)
    in_maps = make_in_maps(q, k, v1, v2, counter_attention_mask)
    nc = _get_nc()
    res = run_bass_kernel_spmd(
        nc, in_maps, core_ids=list(range(NCORES)), trace=TRACE
    )
    LAST_RESULTS = res
    return _finish(res, v1, v2)


# revision 17
# speedup vs baseline: 4.1991x; 1.0166x over previous
"""Trainium2 Bass kernel for dual-attention (DisKT-style) nn module.

Math per (batch, head) with S=1024, dk=64, on-chip in [k, q] layout:
    sT       = (k_h @ q_h^T)          (+ -1e30 on causal-dead diag block)
    E1T      = exp(sT / 8)            (bf16; causally-dead region = 0)
    r1[q]    = sum_k E1T[k, q]        (ones^T @ E1T, PSUM broadcast rows)
    p1       = E1T * rec1[q]          (bf16, DVE 2x mode)
    E2''     = exp(p1) - 1            (bf16; "+1" of every key becomes a
                                       rank-1 vtot fixup applied on host)
    ot1      = [cm*v1 | cm]^T @ E2''  (M=65: row 64 accumulates r2 for free)
    ot2      = [cm*v2]^T   @ E2''
Host: out = (ot[0:64] + vtot) / (1024 + r2);  out[q=0] = 0; transpose.

The counter-mask is folded into the PV weights host-side; the causal-dead
packed layout keeps exp/mul/sub as few big instructions.  Emission is a
2-block-skew software pipeline interleaved at QK-group granularity:
    QK(n,Gi) | PV(n-2, chunk pair i) | exp2/sub(n-1 slotted between groups)
so the in-order PE stream stays continuously fed (p-state stays high) while
ACT (exp1+exp2) runs ~1 group ahead of the PE consumers.

Sharding: data-parallel over batch, B=16 -> 2 per core on 8 cores.
"""

import numpy as np
import ml_dtypes

import concourse.bass as bass
import concourse.mybir as mybir
import concourse.tile as tile
from concourse import bacc
from concourse.bass_utils import run_bass_kernel_spmd

B, S, D, H = 16, 1024, 512, 8
DK = D // H           # 64
NCORES = 8
BLOC = B // NCORES    # 2 batches per core
NCH = S // 128        # 8 k-chunks of 128
F32 = mybir.dt.float32
BF16 = mybir.dt.bfloat16
NPBF16 = ml_dtypes.bfloat16

LIVE = [S - 128 * c for c in range(NCH)]          # live width per chunk
OFF = [sum(LIVE[:c]) for c in range(NCH)]         # packed offset per chunk
PACK = OFF[-1] + LIVE[-1]                         # 4608
# chunk groups sharing one scores-psum tile + one exp1 instruction
GROUPS = [[0], [1], [2], [3], [4, 5], [6, 7]]
NG = len(GROUPS)
# split of the packed exp2/sub into two instructions (chunks 0-1 | 2-7)
CSPLIT = OFF[2]

# knobs that test.py can flip
TRACE = False
LAST_RESULTS = None


def bank_pieces(p0, p1):
    """split [p0, p1) at 512-aligned psum bank boundaries"""
    out = []
    p = p0
    while p < p1:
        end = min(p1, (p // 512 + 1) * 512)
        out.append((p, end))
        p = end
    return out


def build_nc(debug=False):
    nc = bacc.Bacc("TRN2", target_bir_lowering=False, debug=debug)
    AF = mybir.ActivationFunctionType

    qt_d = nc.dram_tensor("qt", [BLOC, H, DK, S], BF16, kind="ExternalInput")
    kt_d = nc.dram_tensor("kt", [BLOC, H, DK, S], BF16, kind="ExternalInput")
    # PV weights, host-transposed to [keys, chunk, dims] per (b, h):
    # w1 dims 0-63 = cm*v1, dim 64 = cm (accumulates r2 in psum row 64)
    w1_d = nc.dram_tensor("w1", [BLOC, H, 128, NCH, 65], BF16, kind="ExternalInput")
    w2_d = nc.dram_tensor("w2", [BLOC, H, 128, NCH, 64], BF16, kind="ExternalInput")
    dmask_d = nc.dram_tensor("dmask", [128, 128], BF16, kind="ExternalInput")
    ident_d = nc.dram_tensor("ident", [128, 128], BF16, kind="ExternalInput")
    ones_d = nc.dram_tensor("onesd", [128, 128], BF16, kind="ExternalInput")
    # raw outputs: [65|64, q] bf16 per (b, h); host normalizes + transposes
    o1_d = nc.dram_tensor("o1t", [BLOC, H, 65, S], BF16, kind="ExternalOutput")
    o2_d = nc.dram_tensor("o2t", [BLOC, H, 64, S], BF16, kind="ExternalOutput")

    with tile.TileContext(nc) as tc:
        with (
            tc.tile_pool(name="consts", bufs=1) as consts,
            tc.tile_pool(name="qk", bufs=4) as qkp,
            tc.tile_pool(name="w", bufs=4) as wp,
            tc.tile_pool(name="e1", bufs=3) as e1p,
            tc.tile_pool(name="tmp", bufs=2) as tmpp,
            tc.tile_pool(name="e2", bufs=4) as e2p,
            tc.tile_pool(name="rc", bufs=2) as rcp,
            tc.tile_pool(name="outs", bufs=2) as outp,
            tc.tile_pool(name="sc_ps", bufs=1, space="PSUM") as sc_psp,
            tc.tile_pool(name="r_ps", bufs=1, space="PSUM") as r_psp,
            tc.tile_pool(name="o1_ps", bufs=1, space="PSUM") as o1_psp,
            tc.tile_pool(name="o2_ps", bufs=1, space="PSUM") as o2_psp,
        ):
            dm_sb = consts.tile([128, 128], BF16)
            nc.sync.dma_start(out=dm_sb, in_=dmask_d[:, :])
            id_sb = consts.tile([128, 128], BF16)
            nc.sync.dma_start(out=id_sb, in_=ident_d[:, :])
            ones_sb = consts.tile([128, 128], BF16)
            nc.sync.dma_start(out=ones_sb, in_=ones_d[:, :])

            NB = BLOC * H
            st = [dict() for _ in range(NB)]

            def dma_in(blk):
                bi, h = divmod(blk, H)
                s = st[blk]
                s["qt"] = qkp.tile([DK, S], BF16, tag="qt", name="qt_sb")
                s["kt"] = qkp.tile([DK, S], BF16, tag="kt", name="kt_sb")
                nc.sync.dma_start(out=s["qt"], in_=qt_d[bi, h])
                nc.sync.dma_start(out=s["kt"], in_=kt_d[bi, h])
                s["w1"] = wp.tile([128, NCH, 65], BF16, tag="w1", name="w1_sb")
                s["w2"] = wp.tile([128, NCH, 64], BF16, tag="w2", name="w2_sb")
                nc.sync.dma_start(out=s["w1"], in_=w1_d[bi, h])
                nc.sync.dma_start(out=s["w2"], in_=w2_d[bi, h])

            def qk_group(blk, gi):
                """scores + causal mask -> exp1 (packed e1) for one group"""
                s = st[blk]
                grp = GROUPS[gi]
                gw = sum(LIVE[c] for c in grp)
                if gi == 0:
                    s["e1"] = e1p.tile([128, PACK], BF16, tag="e1", name="e1_sb")
                    s["r1ps"] = r_psp.tile([128, S], F32, tag="r1", name="r1ps")
                sps = sc_psp.tile([128, gw], F32, tag="sc", name="sps")
                s[f"sc{gi}"] = sps
                loc = 0
                for c in grp:
                    q0 = 128 * c
                    for n0 in range(0, LIVE[c], 512):
                        w = min(512, LIVE[c] - n0)
                        nc.tensor.matmul(
                            sps[:, loc + n0 : loc + n0 + w],
                            lhsT=s["kt"][:, q0 : q0 + 128],
                            rhs=s["qt"][:, q0 + n0 : q0 + n0 + w],
                            start=True,
                            stop=False,
                            skip_group_check=True,
                        )
                    # causal: += I^T @ dmask adds -1e30 on/above diag
                    nc.tensor.matmul(
                        sps[:, loc : loc + 128],
                        lhsT=id_sb,
                        rhs=dm_sb,
                        start=False,
                        stop=True,
                        skip_group_check=True,
                    )
                    loc += LIVE[c]
                o0 = OFF[grp[0]]
                nc.scalar.activation(
                    s["e1"][:, o0 : o0 + gw], sps[:, 0:gw], AF.Exp, scale=0.125
                )

            def r1_group(blk, gi):
                """r1 accumulation for one group's chunks"""
                s = st[blk]
                for c in GROUPS[gi]:
                    q0 = 128 * c
                    for p0, p1 in bank_pieces(q0, S):
                        nc.tensor.matmul(
                            s["r1ps"][:, p0:p1],
                            lhsT=ones_sb,
                            rhs=s["e1"][:, OFF[c] + p0 - q0 : OFF[c] + p1 - q0],
                            start=(c == 0),
                            stop=(c == NCH - 1),
                            skip_group_check=True,
                        )

            def rec1(blk):
                s = st[blk]
                rec1f = rcp.tile([128, S], F32, tag="rec1f")
                nc.vector.reciprocal_approx_fast(out=rec1f, in_=s["r1ps"][:, 0:S])
                rec1b = rcp.tile([128, S], BF16, tag="rec1b")
                nc.vector.tensor_copy(rec1b, rec1f)
                nc.vector.memset(rec1b[:, 0:1], 0.0)
                s["rec1"] = rec1b

            def muls(blk):
                """p1 = e1 * rec1 (bf16, DVE 2x) into tmp"""
                s = st[blk]
                s["tmp"] = tmpp.tile([128, PACK], BF16, tag="tmp", name="tmp_sb")
                for c in range(NCH):
                    q0 = 128 * c
                    nc.vector.tensor_mul(
                        s["tmp"][:, OFF[c] : OFF[c] + LIVE[c]],
                        s["e1"][:, OFF[c] : OFF[c] + LIVE[c]],
                        s["rec1"][:, q0:S],
                    )

            def exp2_part(blk, half):
                s = st[blk]
                x0, x1 = (0, CSPLIT) if half == 0 else (CSPLIT, PACK)
                nc.scalar.activation(s["tmp"][:, x0:x1], s["tmp"][:, x0:x1], AF.Exp)

            def sub_part(blk, half):
                s = st[blk]
                if half == 0:
                    s["e2"] = e2p.tile([128, PACK], BF16, tag="e2", name="e2_sb")
                x0, x1 = (0, CSPLIT) if half == 0 else (CSPLIT, PACK)
                nc.vector.tensor_scalar_add(
                    s["e2"][:, x0:x1], s["tmp"][:, x0:x1], -1.0
                )

            def pv_pair(blk, gi):
                """dense PV sweep for this group's chunks (both outputs)"""
                s = st[blk]
                if gi == 0:
                    s["ot1"] = o1_psp.tile([65, S], F32, tag="ot1", name="ot1ps")
                    s["ot2"] = o2_psp.tile([64, S], F32, tag="ot2", name="ot2ps")
                for c in GROUPS[gi]:
                    q0 = 128 * c
                    for p0, p1 in bank_pieces(q0, S):
                        ee = s["e2"][:, OFF[c] + p0 - q0 : OFF[c] + p1 - q0]
                        nc.tensor.matmul(
                            s["ot1"][:, p0:p1],
                            lhsT=s["w1"][:, c, :],
                            rhs=ee,
                            start=(c == 0),
                            stop=(c == NCH - 1),
                            skip_group_check=True,
                        )
                        nc.tensor.matmul(
                            s["ot2"][:, p0:p1],
                            lhsT=s["w2"][:, c, :],
                            rhs=ee,
                            start=(c == 0),
                            stop=(c == NCH - 1),
                            skip_group_check=True,
                        )

            def dma_out(blk):
                """psum -> sbuf bf16 on DVE (DMA can't read PSUM), then out"""
                bi, h = divmod(blk, H)
                s = st[blk]
                o1_sb = outp.tile([65, S], BF16, tag="o1", name="o1_sb")
                o2_sb = outp.tile([64, S], BF16, tag="o2", name="o2_sb")
                nc.vector.tensor_copy(o1_sb, s["ot1"][:, :])
                nc.vector.tensor_copy(o2_sb, s["ot2"][:, :])
                nc.sync.dma_start(out=o1_d[bi, h], in_=o1_sb)
                nc.sync.dma_start(out=o2_d[bi, h], in_=o2_sb)

            # 3-block-skew pipeline, interleaved at group granularity:
            #   A(n) QK/exp1/r1 | C(n-1) mul/exp2/sub | D(n-3) PV
            # exp1s always lead the ACT queue; exp2 parts slot in behind
            # the exp1 their consumers don't need yet.  PV reads e2 two
            # full periods after the subs, so the elementwise chain is
            # never on the PE critical path.
            dma_in(0)
            for n in range(NB + 3):
                vA = n < NB
                vC = 1 <= n <= NB
                vD = n >= 3
                if n + 1 < NB:
                    dma_in(n + 1)
                if vC:
                    muls(n - 1)
                for gi in range(NG):
                    if vA:
                        qk_group(n, gi)
                        if gi >= 2:
                            r1_group(n, gi - 2)
                    if vD:
                        pv_pair(n - 3, gi)
                # ACT queue: all 6 exp1(n) first, then exp2(n-1) parts.
                # DVE queue: muls(n-1), recip(n), out-casts(n-3), subs(n-1)
                # ordered by expected readiness so nothing blocks the
                # next period's muls.
                if vC:
                    exp2_part(n - 1, 0)
                if vA:
                    r1_group(n, NG - 2)
                    r1_group(n, NG - 1)
                    rec1(n)
                if vC:
                    exp2_part(n - 1, 1)
                if vD:
                    dma_out(n - 3)
                if vC:
                    sub_part(n - 1, 0)
                    sub_part(n - 1, 1)

    nc.compile()
    return nc


_NC_CACHE = None


def _get_nc():
    global _NC_CACHE
    if _NC_CACHE is None:
        _NC_CACHE = build_nc()
    return _NC_CACHE


def make_in_maps(q, k, v1, v2, cm):
    """Full inputs -> per-core input maps (host-side sharding + layout)."""
    q = np.asarray(q, dtype=np.float32).astype(NPBF16)
    k = np.asarray(k, dtype=np.float32).astype(NPBF16)
    v1 = np.asarray(v1, dtype=np.float32)
    v2 = np.asarray(v2, dtype=np.float32)
    cm = np.asarray(cm)

    # additive causal mask for the diagonal block: 0 where k < q else -1e30
    dmask = np.where(
        np.arange(128)[:, None] < np.arange(128)[None, :], 0.0, -1e30
    ).astype(NPBF16)
    ident = np.eye(128, dtype=NPBF16)
    onesd = np.ones((128, 128), NPBF16)

    cml = 1.0 - cm.astype(np.float32)  # [B, S]
    vcm1 = v1 * cml[:, :, None]
    vcm2 = v2 * cml[:, :, None]

    in_maps = []
    for core in range(NCORES):
        b0 = core * BLOC
        qt = np.ascontiguousarray(
            q[b0 : b0 + BLOC].reshape(BLOC, S, H, DK).transpose(0, 2, 3, 1)
        )  # [b, h, dk, s]
        kt = np.ascontiguousarray(
            k[b0 : b0 + BLOC].reshape(BLOC, S, H, DK).transpose(0, 2, 3, 1)
        )
        # [b, h, key-in-chunk, chunk, dim] weight layout (contiguous per key)
        v1s = (
            vcm1[b0 : b0 + BLOC]
            .reshape(BLOC, NCH, 128, H, DK)
            .transpose(0, 3, 2, 1, 4)
        )  # [b, h, 128, c, dk]
        v2s = (
            vcm2[b0 : b0 + BLOC]
            .reshape(BLOC, NCH, 128, H, DK)
            .transpose(0, 3, 2, 1, 4)
        )
        cmls = np.broadcast_to(
            cml[b0 : b0 + BLOC].reshape(BLOC, 1, NCH, 128).transpose(0, 1, 3, 2)[
                :, :, :, :, None
            ],
            (BLOC, H, 128, NCH, 1),
        )
        w1 = np.ascontiguousarray(
            np.concatenate([v1s, cmls], axis=-1).astype(NPBF16)
        )  # [b, h, 128, c, 65]
        w2 = np.ascontiguousarray(v2s.astype(NPBF16))
        in_maps.append(
            dict(qt=qt, kt=kt, w1=w1, w2=w2, dmask=dmask, ident=ident, onesd=onesd)
        )
    return in_maps


def _finish(res, v1, v2):
    o1 = np.concatenate(
        [r["o1t"].astype(np.float32) for r in res.results], axis=0
    )  # [B, H, 65, S]
    o2 = np.concatenate([r["o2t"].astype(np.float32) for r in res.results], axis=0)
    vtot1 = (
        v1.astype(np.float64).reshape(B, S, H, DK).sum(axis=1).astype(np.float32)
    )  # [B, H, dk]
    vtot2 = v2.astype(np.float64).reshape(B, S, H, DK).sum(axis=1).astype(np.float32)
    r2 = 1024.0 + o1[:, :, 64, :]  # [B, H, S]
    out1 = (o1[:, :, 0:64, :] + vtot1[:, :, :, None]) / r2[:, :, None, :]
    out2 = (o2 + vtot2[:, :, :, None]) / r2[:, :, None, :]
    out1 = out1.transpose(0, 3, 1, 2).reshape(B, S, D)
    out2 = out2.transpose(0, 3, 1, 2).reshape(B, S, D)
    out1[:, 0, :] = 0.0
    out2[:, 0, :] = 0.0
    return np.ascontiguousarray(out1), np.ascontiguousarray(out2)


def kernel(q, k, v1, v2, counter_attention_mask):
    global LAST_RESULTS
    v1 = np.asarray(v1, dtype=np.float32)
    v2 = np.asarray(v2, dtype=np.float32)
    in_maps = make_in_maps(q, k, v1, v2, counter_attention_mask)
    nc = _get_nc()
    res = run_bass_kernel_spmd(
        nc, in_maps, core_ids=list(range(NCORES)), trace=TRACE
    )
    LAST_RESULTS = res
    return _finish(res, v1, v2)
